# revision 17
# baseline (speedup 1.0000x reference)
"""Self-contained Trainium2 Bass kernel for nn_CSA_74818330296427.

Computation (see problem reference): QKV projections + per-head scaled
dot-product attention with a kv mask (additive -1e9), a multiplicative
affinity mask, a binary mask (additive -1e9), softmax, PV, and an output
projection. Returns (att_output [4,1024,256], att_weights [4,8,1024,2048]).

Sharding: 8 cores; core c handles batch b = c//2 and query half qh = c%2
(512 queries), all 8 heads. K/V/kv_mask for batch b are replicated on the
two cores of that batch; outputs partition cleanly (no cross-core
reduction).

Per-core dataflow (natural [q-part, k-free] layout):
 - PE-transpose Q/K/V input tiles, project with host-transposed weights
   (scale 1/sqrt(dh) folded into Wq) into QpT/KpT [dout, seq] (bf16) and
   V_aug [k, 8 heads x 65] (bf16; col 64 = ones row for the softmax sums).
 - Mask prep: M = aff*(1-qb)*(1-kv) (f32) and v2 = M*kv_f + qb (bf16);
   scores psum gets  s*1 via QK matmul plus (-1e9*I) @ v2 folded in as a
   second matmul, then one DVE pass e1 = psum * M.
 - exp on ScalarE with fused row-sum accumulation (bf16 out).
 - normalize on GPSIMD: w = e * (1/r)  -> f32 att_weights to DRAM.
 - PV: xbar DMA-transpose of e (bf16) to [k-part, q] tiles; V_aug matmul
   accumulates per-head outputs plus the unnormalized row sums; normalize
   by the ones-row and apply the output projection.
"""

import os
import sys
import time

for _p in ("/opt/trn_rl_repo", "/root/.axon_site/_ro/trn_rl_repo"):
    if os.path.isdir(_p) and _p not in sys.path:
        sys.path.insert(0, _p)

import numpy as np
import ml_dtypes

import bass_rust
import concourse.bass as bass
import concourse.mybir as mybir
from concourse.tile import TileContext
from concourse.bass_utils import run_bass_kernel_spmd

F32 = mybir.dt.float32
BF16 = mybir.dt.bfloat16
Act = mybir.ActivationFunctionType
Alu = mybir.AluOpType

B, SQ_FULL, SKV = 4, 1024, 2048
D, H, DH, DO = 512, 8, 64, 256
SQ = SQ_FULL // 2  # 512 queries per core
N_CORES = 8
NEG = -1e9
SCALE = 1.0 / 8.0  # 1/sqrt(DH)

# ---------------------------------------------------------------------------
# walrus workaround: this build rejects instructions carrying >1 sem-wait.
# Hoist excess waits onto same-engine nops inserted right before.
_WAITNOP_CTR = [0]


def _fix_excess_waits(nc, max_waits=1):
    n = 0
    for f in nc.m.functions:
        for bb in f.blocks:
            out, changed = [], False
            for inst in bb.instructions:
                si = inst.sync_info
                if si is not None and si.on_wait and len(si.on_wait) > max_waits:
                    waits = list(si.on_wait)
                    excess, keep = waits[:-max_waits], waits[-max_waits:]
                    for i in range(0, len(excess), max_waits):
                        _WAITNOP_CTR[0] += 1
                        nop = bass_rust.InstNoOp(
                            name=f"I-waitnop-{_WAITNOP_CTR[0]}", ins=[], outs=[]
                        )
                        nop.engine = inst.engine
                        nop.sync_info = bass_rust.SyncInfo(
                            on_wait=excess[i : i + max_waits], on_update=[]
                        )
                        nc.register_instruction(nop, overwrite=True)
                        out.append(nop)
                    si.on_wait = keep
                    changed = True
                    n += 1
                out.append(inst)
            if changed:
                bb.instructions = out
    return n


def _make_identity(nc, ident):
    nc.gpsimd.memset(ident, 0.0)
    nc.gpsimd.affine_select(
        out=ident,
        in_=ident,
        compare_op=Alu.not_equal,
        fill=1.0,
        base=0,
        pattern=[[-1, ident.shape[0]]],
        channel_multiplier=1,
    )


def build_program(zero_bias=True):
    nc = bass.Bass()

    # ---- per-core DRAM I/O -------------------------------------------------
    q_d = nc.declare_dram_parameter("q", [SQ, D], BF16, isOutput=False)
    k_d = nc.declare_dram_parameter("k", [SKV, D], BF16, isOutput=False)
    v_d = nc.declare_dram_parameter("v", [SKV, D], BF16, isOutput=False)
    aff_d = nc.declare_dram_parameter("aff", [SQ, SKV], BF16, isOutput=False)
    qbf_d = nc.declare_dram_parameter("qbf", [SQ, SKV], BF16, isOutput=False)
    kvz_d = nc.declare_dram_parameter("kvz", [1, SKV], BF16, isOutput=False)
    wqt_d = nc.declare_dram_parameter("wqt", [D, D], BF16, isOutput=False)
    wkt_d = nc.declare_dram_parameter("wkt", [D, D], BF16, isOutput=False)
    wvt_d = nc.declare_dram_parameter("wvt", [D, D], BF16, isOutput=False)
    wot_d = nc.declare_dram_parameter("wot", [D, DO], BF16, isOutput=False)
    bq_d = nc.declare_dram_parameter("bq", [D], F32, isOutput=False)
    bk_d = nc.declare_dram_parameter("bk", [D], F32, isOutput=False)
    bv_d = nc.declare_dram_parameter("bv", [1, D], F32, isOutput=False)
    bo_d = nc.declare_dram_parameter("bo", [1, DO], F32, isOutput=False)
    w_out = nc.declare_dram_parameter("w_out", [H, SQ, SKV], BF16, isOutput=True)
    o_out = nc.declare_dram_parameter("o_out", [SQ, DO], F32, isOutput=True)

    NQ = SQ // 128   # 4 q tiles
    NK = SKV // 128  # 16 k tiles
    NKC = SKV // 512  # 4 k chunks of 512
    ND = D // 128    # 4 din/dout tiles

    with TileContext(nc) as tc:
        with tc.tile_pool(name="persist", bufs=1) as per:
            # persistent tensors
            ident = per.tile([128, 128], F32, tag="ident")
            _make_identity(nc, ident)
            qpt = [per.tile([128, SQ], BF16, tag=f"qpt{i}", name=f"qpt{i}") for i in range(ND)]
            kpt = [per.tile([128, SKV], BF16, tag=f"kpt{i}", name=f"kpt{i}") for i in range(ND)]
            vpa = [per.tile([128, H, DH], BF16, tag=f"vpa{i}", name=f"vpa{i}") for i in range(NK)]
            mm = [per.tile([128, SKV], BF16, tag=f"m{i}", name=f"m{i}") for i in range(NQ)]
            zb = [per.tile([128, SKV], BF16, tag=f"zb{i}", name=f"zb{i}") for i in range(NQ)]
            negm = [per.tile([128, 1], F32, tag=f"negm{i}", name=f"negm{i}") for i in range(NQ)]
            att = [per.tile([128, SQ], BF16, tag=f"att{i}", name=f"att{i}") for i in range(ND)]

            wpool_ctx = tc.tile_pool(name="wpool", bufs=1)
            wpool = wpool_ctx.__enter__()
            wqt_sb = [wpool.tile([128, D], BF16, tag=f"wqt{i}", name=f"wqt{i}") for i in range(ND)]
            wkt_sb = [wpool.tile([128, D], BF16, tag=f"wkt{i}", name=f"wkt{i}") for i in range(ND)]
            wvt_sb = [wpool.tile([128, D], BF16, tag=f"wvt{i}", name=f"wvt{i}") for i in range(ND)]
            wot_sb = [per.tile([128, DO], BF16, tag=f"wot{i}", name=f"wot{i}") for i in range(ND)]
            for i in range(ND):
                nc.sync.dma_start(out=wqt_sb[i], in_=wqt_d[i * 128:(i + 1) * 128, :])
                nc.sync.dma_start(out=wkt_sb[i], in_=wkt_d[i * 128:(i + 1) * 128, :])
                nc.sync.dma_start(out=wvt_sb[i], in_=wvt_d[i * 128:(i + 1) * 128, :])
                nc.sync.dma_start(out=wot_sb[i], in_=wot_d[i * 128:(i + 1) * 128, :])
            bq_sb = per.tile([128, ND], F32, tag="bq")
            nc.sync.dma_start(out=bq_sb, in_=bq_d[:].rearrange("(c p) -> p c", p=128))
            bk_sb = per.tile([128, ND], F32, tag="bk")
            nc.sync.dma_start(out=bk_sb, in_=bk_d[:].rearrange("(c p) -> p c", p=128))
            bv_sb = per.tile([128, D], F32, tag="bv")
            nc.gpsimd.dma_start(out=bv_sb, in_=bv_d[:, :].to_broadcast([128, D]))
            bo_sb = per.tile([128, DO], F32, tag="bo")
            nc.gpsimd.dma_start(out=bo_sb, in_=bo_d[:, :].to_broadcast([128, DO]))
            kvzb = per.tile([128, SKV], BF16, tag="kvzb")
            nc.gpsimd.dma_start(out=kvzb, in_=kvz_d[:, :].to_broadcast([128, SKV]))

            # ---- input loads (issued first), then masks ---------------------
            tin_ctx = tc.tile_pool(name="tin", bufs=1)
            tin = tin_ctx.__enter__()
            qT3 = tin.tile([128, ND, SQ], BF16, tag="qT3", name="qT3")
            nc.sync.dma_start_transpose(qT3, q_d[:, :])
            kT3 = tin.tile([128, ND, SKV], BF16, tag="kT3", name="kT3")
            nc.sync.dma_start_transpose(kT3, k_d[:, :])
            vT3 = tin.tile([128, ND, SKV], BF16, tag="vT3", name="vT3")
            nc.sync.dma_start_transpose(vT3, v_d[:, :])

            with tc.tile_pool(name="mld", bufs=2) as mld, \
                 tc.tile_pool(name="mtmp", bufs=2) as mtmp:
                for qt in range(NQ):
                    aff_t = mld.tile([128, SKV], BF16, tag="aff", name="aff")
                    nc.sync.dma_start(out=aff_t, in_=aff_d[qt * 128:(qt + 1) * 128, :])
                    qbf_t = mld.tile([128, SKV], BF16, tag="qbf", name="qbf")
                    nc.sync.dma_start(out=qbf_t, in_=qbf_d[qt * 128:(qt + 1) * 128, :])
                    qbz = mtmp.tile([128, SKV], BF16, tag="qbz", name="qbz")
                    nc.vector.tensor_scalar(
                        out=qbz, in0=qbf_t, scalar1=-1.0, scalar2=1.0,
                        op0=Alu.mult, op1=Alu.add,
                    )
                    t1 = mtmp.tile([128, SKV], BF16, tag="t1", name="t1")
                    nc.vector.tensor_tensor(out=t1, in0=aff_t, in1=qbz, op=Alu.mult)
                    nc.gpsimd.tensor_tensor(out=mm[qt], in0=t1, in1=kvzb, op=Alu.mult)
                    nc.gpsimd.tensor_tensor(out=zb[qt], in0=qbz, in1=kvzb, op=Alu.mult)
                    zcnt = mtmp.tile([128, 1], F32, tag="zcnt", name="zcnt")
                    nc.vector.tensor_reduce(
                        out=zcnt, in_=zb[qt], axis=mybir.AxisListType.X, op=Alu.add
                    )
                    nc.vector.tensor_scalar(
                        out=negm[qt], in0=zcnt, scalar1=float(-SKV), scalar2=None,
                        op0=Alu.add,
                    )

            # ---- phase B: projections ---------------------------------------
            with tc.tile_pool(name="pps", bufs=2, space="PSUM") as pps:
                def evict(dst, ps, bias_col):
                    if zero_bias:
                        nc.scalar.copy(out=dst, in_=ps)
                    else:
                        nc.vector.tensor_scalar(
                            out=dst, in0=ps, scalar1=bias_col, scalar2=None,
                            op0=Alu.add,
                        )

                for ot in range(ND):
                    ps = pps.tile([128, SQ], F32, tag="pq", name="pq")
                    for it in range(ND):
                        nc.tensor.matmul(
                            ps,
                            wqt_sb[it][:, ot * 128:(ot + 1) * 128],
                            qT3[:, it, :],
                            start=(it == 0),
                            stop=(it == ND - 1),
                        )
                    evict(qpt[ot], ps, bq_sb[:, ot:ot + 1])

                for ot in range(ND):
                    for kc in range(NKC):
                        ps = pps.tile([128, 512], F32, tag="pk", name="pk")
                        for it in range(ND):
                            nc.tensor.matmul(
                                ps,
                                wkt_sb[it][:, ot * 128:(ot + 1) * 128],
                                kT3[:, it, kc * 512:(kc + 1) * 512],
                                start=(it == 0),
                                stop=(it == ND - 1),
                            )
                        evict(kpt[ot][:, kc * 512:(kc + 1) * 512], ps,
                              bk_sb[:, ot:ot + 1])

                for kt in range(NK):
                    ps = pps.tile([128, D], F32, tag="pv", name="pv")
                    for it in range(ND):
                        nc.tensor.matmul(
                            ps,
                            vT3[:, it, kt * 128:(kt + 1) * 128],
                            wvt_sb[it],
                            start=(it == 0),
                            stop=(it == ND - 1),
                        )
                    if zero_bias:
                        nc.scalar.copy(out=vpa[kt][:, :, :].rearrange("p h d -> p (h d)"), in_=ps)
                    else:
                        nc.vector.tensor_tensor(
                            out=vpa[kt][:, :, :],
                            in0=ps.rearrange("p (h d) -> p h d", h=H),
                            in1=bv_sb.rearrange("p (h d) -> p h d", h=H),
                            op=Alu.add,
                        )

            tin_ctx.__exit__(None, None, None)
            wpool_ctx.__exit__(None, None, None)

            # ---- phase 3: attention main loop -------------------------------
            with tc.tile_pool(name="sps", bufs=3, space="PSUM") as sps, \
                 tc.tile_pool(name="ops", bufs=1, space="PSUM") as ops, \
                 tc.tile_pool(name="e1p", bufs=3) as e1p, \
                 tc.tile_pool(name="ebp", bufs=3) as ebp, \
                 tc.tile_pool(name="etp", bufs=3) as etp, \
                 tc.tile_pool(name="wp", bufs=3) as wp, \
                 tc.tile_pool(name="srp", bufs=4) as srp:
                for hp in range(H // 2):
                    h0, h1 = 2 * hp, 2 * hp + 1
                    eTs = {
                        h0: etp.tile([128, NK, NQ, 128], BF16, tag="eT", name="eT0"),
                        h1: etp.tile([128, NK, NQ, 128], BF16, tag="eT", name="eT1"),
                    }
                    for qt in range(NQ):
                        e1s = {}
                        for h in (h0, h1):
                            e1s[h] = e1p.tile([128, SKV], F32, tag="e1",
                                              name="e1")
                        # interleave the two heads' matmuls so they occupy
                        # different PE row groups concurrently
                        for kc in range(NKC):
                            for h in (h0, h1):
                                row0 = 64 * (h % 2)
                                ps = sps.tile([128, 512], F32, tag=f"s{h % 2}",
                                              name="ps")
                                nc.tensor.matmul(
                                    ps,
                                    qpt[hp][row0:row0 + 64,
                                            qt * 128:(qt + 1) * 128],
                                    kpt[hp][row0:row0 + 64,
                                            kc * 512:(kc + 1) * 512],
                                    start=True,
                                    stop=True,
                                )
                                nc.vector.tensor_tensor(
                                    out=e1s[h][:, kc * 512:(kc + 1) * 512],
                                    in0=ps,
                                    in1=mm[qt][:, kc * 512:(kc + 1) * 512],
                                    op=Alu.mult,
                                )
                        for h in (h0, h1):
                            e1 = e1s[h]
                            row0 = 64 * (h % 2)
                            e_bf = ebp.tile([128, SKV], BF16, tag="eb",
                                            name="e_bf")
                            r = srp.tile([128, 1], F32, tag="r", name="r")
                            nc.scalar.activation(
                                out=e_bf, in_=e1, func=Act.Exp, accum_out=r
                            )
                            rt = srp.tile([128, 1], F32, tag="rt", name="rt")
                            nc.vector.tensor_tensor(
                                out=rt, in0=r, in1=negm[qt], op=Alu.add
                            )
                            rc = srp.tile([128, 1], F32, tag="rc", name="rc")
                            nc.vector.reciprocal(out=rc, in_=rt)
                            # mask zeros back in (exp(0)=1 at masked spots)
                            e_z = ebp.tile([128, SKV], BF16, tag="ez",
                                           name="e_z")
                            if h % 2 == 0:
                                nc.vector.tensor_tensor(
                                    out=e_z, in0=e_bf, in1=zb[qt], op=Alu.mult
                                )
                            else:
                                nc.gpsimd.tensor_tensor(
                                    out=e_z, in0=e_bf, in1=zb[qt], op=Alu.mult
                                )
                            wt = wp.tile([128, SKV], BF16, tag="w",
                                         name="wt")
                            nc.vector.tensor_scalar(
                                out=wt, in0=e_z, scalar1=rc, scalar2=None,
                                op0=Alu.mult,
                            )
                            nc.gpsimd.dma_start(
                                out=w_out[h, qt * 128:(qt + 1) * 128, :], in_=wt
                            )
                            nc.sync.dma_start_transpose(eTs[h][:, :, qt, :], wt)

                    for h in (h0, h1):
                        row0 = 64 * (h % 2)
                        pso = ops.tile([DH, SQ], F32, tag=f"pv{h % 2}", name="pso")
                        for kt in range(NK):
                            nc.tensor.matmul(
                                pso,
                                vpa[kt][:, h, :],
                                eTs[h][:, kt, :, :],
                                start=(kt == 0),
                                stop=(kt == NK - 1),
                            )
                        nc.scalar.copy(out=att[hp][row0:row0 + 64, :], in_=pso)

            # ---- output projection ------------------------------------------
            with tc.tile_pool(name="oop", bufs=2, space="PSUM") as oop, \
                 tc.tile_pool(name="wp2", bufs=2) as wp2:
                for qt in range(NQ):
                    ps = oop.tile([128, DO], F32, tag="oo")
                    for ct in range(ND):
                        nc.tensor.matmul(
                            ps,
                            att[ct][:, qt * 128:(qt + 1) * 128],
                            wot_sb[ct],
                            start=(ct == 0),
                            stop=(ct == ND - 1),
                        )
                    osb = wp2.tile([128, DO], F32, tag="osb")
                    if zero_bias:
                        nc.scalar.copy(out=osb, in_=ps)
                    else:
                        nc.vector.tensor_tensor(out=osb, in0=ps, in1=bo_sb, op=Alu.add)
                    nc.sync.dma_start(
                        out=o_out[qt * 128:(qt + 1) * 128, :], in_=osb
                    )

    _fix_excess_waits(nc)
    return nc


_NC_CACHE = {}


def _get_program(zero_bias=True):
    key = ("nc", zero_bias)
    if key not in _NC_CACHE:
        _NC_CACHE[key] = build_program(zero_bias)
    return _NC_CACHE[key]


def _prep_in_maps(Q, K, V, q_aff_mask, q_binary_mask, kv_mask,
                  Wq, bq, Wk, bk, Wv, bv, Wo, bo):
    bf = ml_dtypes.bfloat16
    wqt = np.ascontiguousarray((Wq.astype(np.float32) * SCALE).T).astype(bf)
    wkt = np.ascontiguousarray(Wk.astype(np.float32).T).astype(bf)
    wvt = np.ascontiguousarray(Wv.astype(np.float32).T).astype(bf)
    wot = np.ascontiguousarray(Wo.astype(np.float32).T).astype(bf)
    bqs = (bq.astype(np.float32) * SCALE)
    bkf = bk.astype(np.float32)
    bvf = bv.astype(np.float32).reshape(1, D)
    bof = bo.astype(np.float32).reshape(1, DO)

    in_maps = []
    for c in range(N_CORES):
        b, qh = c // 2, c % 2
        qs = slice(qh * SQ, (qh + 1) * SQ)
        in_maps.append({
            "q": Q[b, qs, :].astype(bf),
            "k": K[b].astype(bf),
            "v": V[b].astype(bf),
            "aff": q_aff_mask[b, qs, :].astype(bf),
            "qbf": q_binary_mask[b, qs, :].astype(bf),
            "kvz": (1 - kv_mask[b, :, 0]).astype(bf).reshape(1, SKV),
            "wqt": wqt, "wkt": wkt, "wvt": wvt, "wot": wot,
            "bq": bqs, "bk": bkf, "bv": bvf, "bo": bof,
        })
    return in_maps


def run(in_maps, trace=False, zero_bias=True, **kw):
    nc = _get_program(zero_bias)
    return run_bass_kernel_spmd(nc, in_maps, list(range(N_CORES)), trace=trace, **kw)


def _all_zero_bias(inputs):
    return all(
        not np.any(np.asarray(inputs[k])) for k in ("bq", "bk", "bv", "bo")
    )


def kernel(**inputs):
    in_maps = _prep_in_maps(**inputs)
    zb_flag = _all_zero_bias(inputs)
    res = run(in_maps, zero_bias=zb_flag)
    att_output = np.empty((B, SQ_FULL, DO), np.float32)
    att_weights = np.empty((B, H, SQ_FULL, SKV), np.float32)
    for c in range(N_CORES):
        b, qh = c // 2, c % 2
        qs = slice(qh * SQ, (qh + 1) * SQ)
        att_output[b, qs, :] = res.results[c]["o_out"]
        att_weights[b, :, qs, :] = res.results[c]["w_out"].astype(np.float32)
    return att_output, att_weights


if __name__ == "__main__":
    t0 = time.time()
    _get_program()
    print("build s:", time.time() - t0)


# revision 22
# speedup vs baseline: 1.0481x; 1.0481x over previous
"""Self-contained Trainium2 Bass kernel for nn_CSA_74818330296427.

Computation (see problem reference): QKV projections + per-head scaled
dot-product attention with a kv mask (additive -1e9), a multiplicative
affinity mask, a binary mask (additive -1e9), softmax, PV, and an output
projection. Returns (att_output [4,1024,256], att_weights [4,8,1024,2048]).

Sharding: 8 cores; core c handles batch b = c//2 and query half qh = c%2
(512 queries), all 8 heads. K/V/kv_mask for batch b are replicated on the
two cores of that batch; outputs partition cleanly (no cross-core
reduction).

Per-core dataflow (natural [q-part, k-free] layout):
 - PE-transpose Q/K/V input tiles, project with host-transposed weights
   (scale 1/sqrt(dh) folded into Wq) into QpT/KpT [dout, seq] (bf16) and
   V_aug [k, 8 heads x 65] (bf16; col 64 = ones row for the softmax sums).
 - Mask prep: M = aff*(1-qb)*(1-kv) (f32) and v2 = M*kv_f + qb (bf16);
   scores psum gets  s*1 via QK matmul plus (-1e9*I) @ v2 folded in as a
   second matmul, then one DVE pass e1 = psum * M.
 - exp on ScalarE with fused row-sum accumulation (bf16 out).
 - normalize on GPSIMD: w = e * (1/r)  -> f32 att_weights to DRAM.
 - PV: xbar DMA-transpose of e (bf16) to [k-part, q] tiles; V_aug matmul
   accumulates per-head outputs plus the unnormalized row sums; normalize
   by the ones-row and apply the output projection.
"""

import os
import sys
import time

for _p in ("/opt/trn_rl_repo", "/root/.axon_site/_ro/trn_rl_repo"):
    if os.path.isdir(_p) and _p not in sys.path:
        sys.path.insert(0, _p)

import numpy as np
import ml_dtypes

import bass_rust
import concourse.bass as bass
import concourse.mybir as mybir
from concourse.tile import TileContext
from concourse.bass_utils import run_bass_kernel_spmd

F32 = mybir.dt.float32
BF16 = mybir.dt.bfloat16
Act = mybir.ActivationFunctionType
Alu = mybir.AluOpType

B, SQ_FULL, SKV = 4, 1024, 2048
D, H, DH, DO = 512, 8, 64, 256
SQ = SQ_FULL // 2  # 512 queries per core
N_CORES = 8
NEG = -1e9
SCALE = 1.0 / 8.0  # 1/sqrt(DH)

# ---------------------------------------------------------------------------
# walrus workaround: this build rejects instructions carrying >1 sem-wait.
# Hoist excess waits onto same-engine nops inserted right before.
_WAITNOP_CTR = [0]


def _fix_excess_waits(nc, max_waits=1):
    n = 0
    for f in nc.m.functions:
        for bb in f.blocks:
            out, changed = [], False
            for inst in bb.instructions:
                si = inst.sync_info
                if si is not None and si.on_wait and len(si.on_wait) > max_waits:
                    waits = list(si.on_wait)
                    excess, keep = waits[:-max_waits], waits[-max_waits:]
                    for i in range(0, len(excess), max_waits):
                        _WAITNOP_CTR[0] += 1
                        nop = bass_rust.InstNoOp(
                            name=f"I-waitnop-{_WAITNOP_CTR[0]}", ins=[], outs=[]
                        )
                        nop.engine = inst.engine
                        nop.sync_info = bass_rust.SyncInfo(
                            on_wait=excess[i : i + max_waits], on_update=[]
                        )
                        nc.register_instruction(nop, overwrite=True)
                        out.append(nop)
                    si.on_wait = keep
                    changed = True
                    n += 1
                out.append(inst)
            if changed:
                bb.instructions = out
    return n


def _make_identity(nc, ident):
    nc.gpsimd.memset(ident, 0.0)
    nc.gpsimd.affine_select(
        out=ident,
        in_=ident,
        compare_op=Alu.not_equal,
        fill=1.0,
        base=0,
        pattern=[[-1, ident.shape[0]]],
        channel_multiplier=1,
    )


def build_program(zero_bias=True):
    nc = bass.Bass()

    # ---- per-core DRAM I/O -------------------------------------------------
    q_d = nc.declare_dram_parameter("q", [SQ, D], BF16, isOutput=False)
    k_d = nc.declare_dram_parameter("k", [SKV, D], BF16, isOutput=False)
    v_d = nc.declare_dram_parameter("v", [SKV, D], BF16, isOutput=False)
    aff_d = nc.declare_dram_parameter("aff", [SQ, SKV], BF16, isOutput=False)
    qbz_d = nc.declare_dram_parameter("qbz", [SQ, SKV], BF16, isOutput=False)
    kvz_d = nc.declare_dram_parameter("kvz", [1, SKV], BF16, isOutput=False)
    wqt_d = nc.declare_dram_parameter("wqt", [D, D], BF16, isOutput=False)
    wkt_d = nc.declare_dram_parameter("wkt", [D, D], BF16, isOutput=False)
    wvt_d = nc.declare_dram_parameter("wvt", [D, D], BF16, isOutput=False)
    wot_d = nc.declare_dram_parameter("wot", [D, DO], BF16, isOutput=False)
    bq_d = nc.declare_dram_parameter("bq", [D], F32, isOutput=False)
    bk_d = nc.declare_dram_parameter("bk", [D], F32, isOutput=False)
    bv_d = nc.declare_dram_parameter("bv", [1, D], F32, isOutput=False)
    bo_d = nc.declare_dram_parameter("bo", [1, DO], F32, isOutput=False)
    w_out = nc.declare_dram_parameter("w_out", [H, SQ, SKV], BF16, isOutput=True)
    o_out = nc.declare_dram_parameter("o_out", [SQ, DO], F32, isOutput=True)

    NQ = SQ // 128   # 4 q tiles
    NK = SKV // 128  # 16 k tiles
    NKC = SKV // 512  # 4 k chunks of 512
    ND = D // 128    # 4 din/dout tiles

    with TileContext(nc) as tc:
        with tc.tile_pool(name="persist", bufs=1) as per:
            # persistent tensors
            ident = per.tile([128, 128], F32, tag="ident")
            _make_identity(nc, ident)
            qpt = [per.tile([128, SQ], BF16, tag=f"qpt{i}", name=f"qpt{i}") for i in range(ND)]
            kpt = [per.tile([128, SKV], BF16, tag=f"kpt{i}", name=f"kpt{i}") for i in range(ND)]
            vpa = [per.tile([128, H, DH], BF16, tag=f"vpa{i}", name=f"vpa{i}") for i in range(NK)]
            mm = [per.tile([128, SKV], BF16, tag=f"m{i}", name=f"m{i}") for i in range(NQ)]
            zb = [per.tile([128, SKV], BF16, tag=f"zb{i}", name=f"zb{i}") for i in range(NQ)]
            negm = [per.tile([128, 1], F32, tag=f"negm{i}", name=f"negm{i}") for i in range(NQ)]
            att = [per.tile([128, SQ], BF16, tag=f"att{i}", name=f"att{i}") for i in range(ND)]

            wot_sb = [per.tile([128, DO], BF16, tag=f"wot{i}", name=f"wot{i}") for i in range(ND)]
            for i in range(ND):
                nc.sync.dma_start(out=wot_sb[i], in_=wot_d[i * 128:(i + 1) * 128, :])
            bq_sb = per.tile([128, ND], F32, tag="bq")
            nc.sync.dma_start(out=bq_sb, in_=bq_d[:].rearrange("(c p) -> p c", p=128))
            bk_sb = per.tile([128, ND], F32, tag="bk")
            nc.sync.dma_start(out=bk_sb, in_=bk_d[:].rearrange("(c p) -> p c", p=128))
            bv_sb = per.tile([128, D], F32, tag="bv")
            nc.gpsimd.dma_start(out=bv_sb, in_=bv_d[:, :].to_broadcast([128, D]))
            bo_sb = per.tile([128, DO], F32, tag="bo")
            nc.gpsimd.dma_start(out=bo_sb, in_=bo_d[:, :].to_broadcast([128, DO]))
            kvzb = per.tile([128, SKV], BF16, tag="kvzb")
            nc.gpsimd.dma_start(out=kvzb, in_=kvz_d[:, :].to_broadcast([128, SKV]))

            # ---- mask pools (outlive setup; used inside main loop) ----------
            mld_ctx = tc.tile_pool(name="mld", bufs=2)
            mld = mld_ctx.__enter__()
            mtmp_ctx = tc.tile_pool(name="mtmp", bufs=4)
            mtmp = mtmp_ctx.__enter__()

            # ---- input loads (issued first), then masks ---------------------
            tin_ctx = tc.tile_pool(name="tin", bufs=1)
            tin = tin_ctx.__enter__()
            wqt_sb = [tin.tile([128, D], BF16, tag=f"wqt{i}", name=f"wqt{i}") for i in range(ND)]
            wkt_sb = [tin.tile([128, D], BF16, tag=f"wkt{i}", name=f"wkt{i}") for i in range(ND)]
            wvt_sb = [tin.tile([128, D], BF16, tag=f"wvt{i}", name=f"wvt{i}") for i in range(ND)]
            for i in range(ND):
                nc.sync.dma_start(out=wqt_sb[i], in_=wqt_d[i * 128:(i + 1) * 128, :])
                nc.sync.dma_start(out=wkt_sb[i], in_=wkt_d[i * 128:(i + 1) * 128, :])
                nc.sync.dma_start(out=wvt_sb[i], in_=wvt_d[i * 128:(i + 1) * 128, :])
            qT3 = tin.tile([128, ND, SQ], BF16, tag="qT3", name="qT3")
            nc.sync.dma_start_transpose(qT3, q_d[:, :])
            kT3 = tin.tile([128, ND, SKV], BF16, tag="kT3", name="kT3")
            nc.sync.dma_start_transpose(kT3, k_d[:, :])
            vT3 = tin.tile([128, ND, SKV], BF16, tag="vT3", name="vT3")
            nc.scalar.dma_start_transpose(vT3, v_d[:, :])

            mask_done = [False] * NQ

            def build_mask(qt):
                if mask_done[qt]:
                    return
                mask_done[qt] = True
                aff_t = mld.tile([128, SKV], BF16, tag="aff", name="aff")
                nc.gpsimd.dma_start(out=aff_t, in_=aff_d[qt * 128:(qt + 1) * 128, :])
                qbz_t = mld.tile([128, SKV], BF16, tag="qbz", name="qbz")
                nc.gpsimd.dma_start(out=qbz_t, in_=qbz_d[qt * 128:(qt + 1) * 128, :])
                eng = nc.vector if qt % 2 == 0 else nc.gpsimd
                eng.tensor_tensor(out=zb[qt], in0=qbz_t, in1=kvzb, op=Alu.mult)
                eng2 = nc.gpsimd if qt % 2 == 0 else nc.vector
                eng2.tensor_tensor(out=mm[qt], in0=aff_t, in1=zb[qt], op=Alu.mult)
                zcnt = mtmp.tile([128, 1], F32, tag="zcnt", name="zcnt")
                nc.vector.tensor_reduce(
                    out=zcnt, in_=zb[qt], axis=mybir.AxisListType.X, op=Alu.add
                )
                nc.vector.tensor_scalar(
                    out=negm[qt], in0=zcnt, scalar1=float(-SKV), scalar2=None,
                    op0=Alu.add,
                )

            # ---- phase B: projections ---------------------------------------
            with tc.tile_pool(name="pps", bufs=2, space="PSUM") as pps:
                def evict(dst, ps, bias_col):
                    if zero_bias:
                        nc.scalar.copy(out=dst, in_=ps)
                    else:
                        nc.vector.tensor_scalar(
                            out=dst, in0=ps, scalar1=bias_col, scalar2=None,
                            op0=Alu.add,
                        )

                for ot in range(ND):
                    ps = pps.tile([128, SQ], F32, tag="pq", name="pq")
                    for it in range(ND):
                        nc.tensor.matmul(
                            ps,
                            wqt_sb[it][:, ot * 128:(ot + 1) * 128],
                            qT3[:, it, :],
                            start=(it == 0),
                            stop=(it == ND - 1),
                        )
                    evict(qpt[ot], ps, bq_sb[:, ot:ot + 1])

                for ot in range(ND):
                    for kc in range(NKC):
                        ps = pps.tile([128, 512], F32, tag="pk", name="pk")
                        for it in range(ND):
                            nc.tensor.matmul(
                                ps,
                                wkt_sb[it][:, ot * 128:(ot + 1) * 128],
                                kT3[:, it, kc * 512:(kc + 1) * 512],
                                start=(it == 0),
                                stop=(it == ND - 1),
                            )
                        evict(kpt[ot][:, kc * 512:(kc + 1) * 512], ps,
                              bk_sb[:, ot:ot + 1])

                for kt in range(NK):
                    ps = pps.tile([128, D], F32, tag="pv", name="pv")
                    for it in range(ND):
                        nc.tensor.matmul(
                            ps,
                            vT3[:, it, kt * 128:(kt + 1) * 128],
                            wvt_sb[it],
                            start=(it == 0),
                            stop=(it == ND - 1),
                        )
                    if zero_bias:
                        nc.scalar.copy(out=vpa[kt][:, :, :].rearrange("p h d -> p (h d)"), in_=ps)
                    else:
                        nc.vector.tensor_tensor(
                            out=vpa[kt][:, :, :],
                            in0=ps.rearrange("p (h d) -> p h d", h=H),
                            in1=bv_sb.rearrange("p (h d) -> p h d", h=H),
                            op=Alu.add,
                        )

            tin_ctx.__exit__(None, None, None)

            # ---- phase 3: attention main loop -------------------------------
            with tc.tile_pool(name="sps", bufs=3, space="PSUM") as sps, \
                 tc.tile_pool(name="ops", bufs=1, space="PSUM") as ops, \
                 tc.tile_pool(name="e1p", bufs=3) as e1p, \
                 tc.tile_pool(name="ebp", bufs=3) as ebp, \
                 tc.tile_pool(name="etp", bufs=3) as etp, \
                 tc.tile_pool(name="wp", bufs=2) as wp, \
                 tc.tile_pool(name="srp", bufs=4) as srp:
                for hp in range(H // 2):
                    h0, h1 = 2 * hp, 2 * hp + 1
                    eTs = {
                        h0: etp.tile([128, NK, NQ, 128], BF16, tag="eT", name="eT0"),
                        h1: etp.tile([128, NK, NQ, 128], BF16, tag="eT", name="eT1"),
                    }
                    for qt in range(NQ):
                        build_mask(qt)
                        e1s = {}
                        for h in (h0, h1):
                            e1s[h] = e1p.tile([128, SKV], F32, tag="e1",
                                              name="e1")
                        # interleave the two heads' matmuls so they occupy
                        # different PE row groups concurrently
                        for kc in range(NKC):
                            for h in (h0, h1):
                                row0 = 64 * (h % 2)
                                ps = sps.tile([128, 512], F32, tag=f"s{h % 2}",
                                              name="ps")
                                nc.tensor.matmul(
                                    ps,
                                    qpt[hp][row0:row0 + 64,
                                            qt * 128:(qt + 1) * 128],
                                    kpt[hp][row0:row0 + 64,
                                            kc * 512:(kc + 1) * 512],
                                    start=True,
                                    stop=True,
                                )
                                nc.vector.tensor_tensor(
                                    out=e1s[h][:, kc * 512:(kc + 1) * 512],
                                    in0=ps,
                                    in1=mm[qt][:, kc * 512:(kc + 1) * 512],
                                    op=Alu.mult,
                                )
                        for h in (h0, h1):
                            e1 = e1s[h]
                            row0 = 64 * (h % 2)
                            e_bf = ebp.tile([128, SKV], BF16, tag="eb",
                                            name="e_bf")
                            rparts = []
                            for kc in range(NKC):
                                rp = srp.tile([128, 1], F32, tag="rp", name="rp")
                                nc.scalar.activation(
                                    out=e_bf[:, kc * 512:(kc + 1) * 512],
                                    in_=e1[:, kc * 512:(kc + 1) * 512],
                                    func=Act.Exp, accum_out=rp,
                                )
                                rparts.append(rp)
                            r01 = srp.tile([128, 1], F32, tag="r01", name="r01")
                            nc.vector.tensor_tensor(
                                out=r01, in0=rparts[0], in1=rparts[1], op=Alu.add
                            )
                            r23 = srp.tile([128, 1], F32, tag="r23", name="r23")
                            nc.vector.tensor_tensor(
                                out=r23, in0=rparts[2], in1=rparts[3], op=Alu.add
                            )
                            r03 = srp.tile([128, 1], F32, tag="r03", name="r03")
                            nc.vector.tensor_tensor(
                                out=r03, in0=r01, in1=r23, op=Alu.add
                            )
                            rt = srp.tile([128, 1], F32, tag="rt", name="rt")
                            nc.vector.tensor_tensor(
                                out=rt, in0=r03, in1=negm[qt], op=Alu.add
                            )
                            rc = srp.tile([128, 1], F32, tag="rc", name="rc")
                            nc.vector.reciprocal(out=rc, in_=rt)
                            # mask zeros back in (exp(0)=1 at masked spots)
                            e_z = ebp.tile([128, SKV], BF16, tag="ez",
                                           name="e_z")
                            if h % 2 == 0:
                                nc.vector.tensor_tensor(
                                    out=e_z, in0=e_bf, in1=zb[qt], op=Alu.mult
                                )
                            else:
                                nc.gpsimd.tensor_tensor(
                                    out=e_z, in0=e_bf, in1=zb[qt], op=Alu.mult
                                )
                            wt = wp.tile([128, SKV], BF16, tag="w",
                                         name="wt")
                            nc.vector.tensor_scalar(
                                out=wt, in0=e_z, scalar1=rc, scalar2=None,
                                op0=Alu.mult,
                            )
                            nc.gpsimd.dma_start(
                                out=w_out[h, qt * 128:(qt + 1) * 128, :], in_=wt
                            )
                            nc.sync.dma_start_transpose(eTs[h][:, :, qt, :], wt)

                    for h in (h0, h1):
                        row0 = 64 * (h % 2)
                        pso = ops.tile([DH, SQ], F32, tag=f"pv{h % 2}", name="pso")
                        for kt in range(NK):
                            nc.tensor.matmul(
                                pso,
                                vpa[kt][:, h, :],
                                eTs[h][:, kt, :, :],
                                start=(kt == 0),
                                stop=(kt == NK - 1),
                            )
                        nc.scalar.copy(out=att[hp][row0:row0 + 64, :], in_=pso)

            mtmp_ctx.__exit__(None, None, None)
            mld_ctx.__exit__(None, None, None)

            # ---- output projection ------------------------------------------
            with tc.tile_pool(name="oop", bufs=2, space="PSUM") as oop, \
                 tc.tile_pool(name="wp2", bufs=2) as wp2:
                for qt in range(NQ):
                    ps = oop.tile([128, DO], F32, tag="oo")
                    for ct in range(ND):
                        nc.tensor.matmul(
                            ps,
                            att[ct][:, qt * 128:(qt + 1) * 128],
                            wot_sb[ct],
                            start=(ct == 0),
                            stop=(ct == ND - 1),
                        )
                    osb = wp2.tile([128, DO], F32, tag="osb")
                    if zero_bias:
                        nc.scalar.copy(out=osb, in_=ps)
                    else:
                        nc.vector.tensor_tensor(out=osb, in0=ps, in1=bo_sb, op=Alu.add)
                    nc.sync.dma_start(
                        out=o_out[qt * 128:(qt + 1) * 128, :], in_=osb
                    )

    _fix_excess_waits(nc)
    return nc


_NC_CACHE = {}


def _get_program(zero_bias=True):
    key = ("nc", zero_bias)
    if key not in _NC_CACHE:
        _NC_CACHE[key] = build_program(zero_bias)
    return _NC_CACHE[key]


def _prep_in_maps(Q, K, V, q_aff_mask, q_binary_mask, kv_mask,
                  Wq, bq, Wk, bk, Wv, bv, Wo, bo):
    bf = ml_dtypes.bfloat16
    wqt = np.ascontiguousarray((Wq.astype(np.float32) * SCALE).T).astype(bf)
    wkt = np.ascontiguousarray(Wk.astype(np.float32).T).astype(bf)
    wvt = np.ascontiguousarray(Wv.astype(np.float32).T).astype(bf)
    wot = np.ascontiguousarray(Wo.astype(np.float32).T).astype(bf)
    bqs = (bq.astype(np.float32) * SCALE)
    bkf = bk.astype(np.float32)
    bvf = bv.astype(np.float32).reshape(1, D)
    bof = bo.astype(np.float32).reshape(1, DO)

    in_maps = []
    for c in range(N_CORES):
        b, qh = c // 2, c % 2
        qs = slice(qh * SQ, (qh + 1) * SQ)
        in_maps.append({
            "q": Q[b, qs, :].astype(bf),
            "k": K[b].astype(bf),
            "v": V[b].astype(bf),
            "aff": q_aff_mask[b, qs, :].astype(bf),
            "qbz": (1 - q_binary_mask[b, qs, :]).astype(bf),
            "kvz": (1 - kv_mask[b, :, 0]).astype(bf).reshape(1, SKV),
            "wqt": wqt, "wkt": wkt, "wvt": wvt, "wot": wot,
            "bq": bqs, "bk": bkf, "bv": bvf, "bo": bof,
        })
    return in_maps


def run(in_maps, trace=False, zero_bias=True, **kw):
    nc = _get_program(zero_bias)
    return run_bass_kernel_spmd(nc, in_maps, list(range(N_CORES)), trace=trace, **kw)


def _all_zero_bias(inputs):
    return all(
        not np.any(np.asarray(inputs[k])) for k in ("bq", "bk", "bv", "bo")
    )


def kernel(**inputs):
    in_maps = _prep_in_maps(**inputs)
    zb_flag = _all_zero_bias(inputs)
    res = run(in_maps, zero_bias=zb_flag)
    att_output = np.empty((B, SQ_FULL, DO), np.float32)
    att_weights = np.empty((B, H, SQ_FULL, SKV), np.float32)
    for c in range(N_CORES):
        b, qh = c // 2, c % 2
        qs = slice(qh * SQ, (qh + 1) * SQ)
        att_output[b, qs, :] = res.results[c]["o_out"]
        att_weights[b, :, qs, :] = res.results[c]["w_out"].astype(np.float32)
    return att_output, att_weights


if __name__ == "__main__":
    t0 = time.time()
    _get_program()
    print("build s:", time.time() - t0)


# revision 24
# speedup vs baseline: 1.0606x; 1.0119x over previous
"""Self-contained Trainium2 Bass kernel for nn_CSA_74818330296427.

Computation (see problem reference): QKV projections + per-head scaled
dot-product attention with a kv mask (additive -1e9), a multiplicative
affinity mask, a binary mask (additive -1e9), softmax, PV, and an output
projection. Returns (att_output [4,1024,256], att_weights [4,8,1024,2048]).

Sharding: 8 cores; core c handles batch b = c//2 and query half qh = c%2
(512 queries), all 8 heads. K/V/kv_mask for batch b are replicated on the
two cores of that batch; outputs partition cleanly (no cross-core
reduction).

Per-core dataflow (natural [q-part, k-free] layout):
 - PE-transpose Q/K/V input tiles, project with host-transposed weights
   (scale 1/sqrt(dh) folded into Wq) into QpT/KpT [dout, seq] (bf16) and
   V_aug [k, 8 heads x 65] (bf16; col 64 = ones row for the softmax sums).
 - Mask prep: M = aff*(1-qb)*(1-kv) (f32) and v2 = M*kv_f + qb (bf16);
   scores psum gets  s*1 via QK matmul plus (-1e9*I) @ v2 folded in as a
   second matmul, then one DVE pass e1 = psum * M.
 - exp on ScalarE with fused row-sum accumulation (bf16 out).
 - normalize on GPSIMD: w = e * (1/r)  -> f32 att_weights to DRAM.
 - PV: xbar DMA-transpose of e (bf16) to [k-part, q] tiles; V_aug matmul
   accumulates per-head outputs plus the unnormalized row sums; normalize
   by the ones-row and apply the output projection.
"""

import os
import sys
import time

for _p in ("/opt/trn_rl_repo", "/root/.axon_site/_ro/trn_rl_repo"):
    if os.path.isdir(_p) and _p not in sys.path:
        sys.path.insert(0, _p)

import numpy as np
import ml_dtypes

import bass_rust
import concourse.bass as bass
import concourse.mybir as mybir
from concourse.tile import TileContext
from concourse.bass_utils import run_bass_kernel_spmd

F32 = mybir.dt.float32
BF16 = mybir.dt.bfloat16
Act = mybir.ActivationFunctionType
Alu = mybir.AluOpType

B, SQ_FULL, SKV = 4, 1024, 2048
D, H, DH, DO = 512, 8, 64, 256
SQ = SQ_FULL // 2  # 512 queries per core
N_CORES = 8
NEG = -1e9
SCALE = 1.0 / 8.0  # 1/sqrt(DH)

# ---------------------------------------------------------------------------
# walrus workaround: this build rejects instructions carrying >1 sem-wait.
# Hoist excess waits onto same-engine nops inserted right before.
_WAITNOP_CTR = [0]


def _fix_excess_waits(nc, max_waits=1):
    n = 0
    for f in nc.m.functions:
        for bb in f.blocks:
            out, changed = [], False
            for inst in bb.instructions:
                si = inst.sync_info
                if si is not None and si.on_wait and len(si.on_wait) > max_waits:
                    waits = list(si.on_wait)
                    excess, keep = waits[:-max_waits], waits[-max_waits:]
                    for i in range(0, len(excess), max_waits):
                        _WAITNOP_CTR[0] += 1
                        nop = bass_rust.InstNoOp(
                            name=f"I-waitnop-{_WAITNOP_CTR[0]}", ins=[], outs=[]
                        )
                        nop.engine = inst.engine
                        nop.sync_info = bass_rust.SyncInfo(
                            on_wait=excess[i : i + max_waits], on_update=[]
                        )
                        nc.register_instruction(nop, overwrite=True)
                        out.append(nop)
                    si.on_wait = keep
                    changed = True
                    n += 1
                out.append(inst)
            if changed:
                bb.instructions = out
    return n


def _make_identity(nc, ident):
    nc.gpsimd.memset(ident, 0.0)
    nc.gpsimd.affine_select(
        out=ident,
        in_=ident,
        compare_op=Alu.not_equal,
        fill=1.0,
        base=0,
        pattern=[[-1, ident.shape[0]]],
        channel_multiplier=1,
    )


def build_program(zero_bias=True):
    nc = bass.Bass()
    NQ = SQ // 128   # 4 q tiles
    NK = SKV // 128  # 16 k tiles
    NKC = SKV // 512  # 4 k chunks of 512
    ND = D // 128    # 4 din/dout tiles

    # ---- per-core DRAM I/O -------------------------------------------------
    q_d = nc.declare_dram_parameter("q", [SQ, D], BF16, isOutput=False)
    k_d = nc.declare_dram_parameter("k", [SKV, D], BF16, isOutput=False)
    v_d = nc.declare_dram_parameter("v", [SKV, D], BF16, isOutput=False)
    aff_d = nc.declare_dram_parameter("aff", [SQ, SKV], BF16, isOutput=False)
    qbz_d = nc.declare_dram_parameter("qbz", [SQ, SKV], BF16, isOutput=False)
    kvz_d = nc.declare_dram_parameter("kvz", [1, SKV], BF16, isOutput=False)
    wqt_d = nc.declare_dram_parameter("wqt", [D, D], BF16, isOutput=False)
    wkt_d = nc.declare_dram_parameter("wkt", [D, D], BF16, isOutput=False)
    wvt_d = nc.declare_dram_parameter("wvt", [D, D], BF16, isOutput=False)
    wot_d = nc.declare_dram_parameter("wot", [D, DO], BF16, isOutput=False)
    bq_d = nc.declare_dram_parameter("bq", [128, ND], F32, isOutput=False)
    bk_d = nc.declare_dram_parameter("bk", [128, ND], F32, isOutput=False)
    bv_d = nc.declare_dram_parameter("bv", [1, D], F32, isOutput=False)
    bo_d = nc.declare_dram_parameter("bo", [1, DO], F32, isOutput=False)
    w_out = nc.declare_dram_parameter("w_out", [H, SQ, SKV], BF16, isOutput=True)
    o_out = nc.declare_dram_parameter("o_out", [SQ, DO], F32, isOutput=True)

    with TileContext(nc) as tc:
        with tc.tile_pool(name="persist", bufs=1) as per:
            # persistent tensors
            ident = per.tile([128, 128], F32, tag="ident")
            _make_identity(nc, ident)
            qpt = [per.tile([128, SQ], BF16, tag=f"qpt{i}", name=f"qpt{i}") for i in range(ND)]
            kpt = [per.tile([128, SKV], BF16, tag=f"kpt{i}", name=f"kpt{i}") for i in range(ND)]
            vpa = [per.tile([128, H, DH], BF16, tag=f"vpa{i}", name=f"vpa{i}") for i in range(NK)]
            mm = [per.tile([128, SKV], BF16, tag=f"m{i}", name=f"m{i}") for i in range(NQ)]
            zb = [per.tile([128, SKV], BF16, tag=f"zb{i}", name=f"zb{i}") for i in range(NQ)]
            negm = [per.tile([128, 1], F32, tag=f"negm{i}", name=f"negm{i}") for i in range(NQ)]
            att = [per.tile([128, SQ], BF16, tag=f"att{i}", name=f"att{i}") for i in range(ND)]

            wot_sb = [per.tile([128, DO], BF16, tag=f"wot{i}", name=f"wot{i}") for i in range(ND)]
            for i in range(ND):
                nc.sync.dma_start(out=wot_sb[i], in_=wot_d[i * 128:(i + 1) * 128, :])
            if not zero_bias:
                bq_sb = per.tile([128, ND], F32, tag="bq")
                nc.sync.dma_start(out=bq_sb, in_=bq_d[:, :])
                bk_sb = per.tile([128, ND], F32, tag="bk")
                nc.sync.dma_start(out=bk_sb, in_=bk_d[:, :])
                bv_sb = per.tile([128, D], F32, tag="bv")
                nc.gpsimd.dma_start(out=bv_sb, in_=bv_d[:, :].to_broadcast([128, D]))
                bo_sb = per.tile([128, DO], F32, tag="bo")
                nc.gpsimd.dma_start(out=bo_sb, in_=bo_d[:, :].to_broadcast([128, DO]))
            else:
                bq_sb = bk_sb = bv_sb = bo_sb = None
            kvzb = per.tile([128, SKV], BF16, tag="kvzb")
            nc.gpsimd.dma_start(out=kvzb, in_=kvz_d[:, :].to_broadcast([128, SKV]))

            # ---- mask pools (outlive setup; used inside main loop) ----------
            mld_ctx = tc.tile_pool(name="mld", bufs=2)
            mld = mld_ctx.__enter__()
            mtmp_ctx = tc.tile_pool(name="mtmp", bufs=4)
            mtmp = mtmp_ctx.__enter__()

            # ---- input loads (issued first), then masks ---------------------
            tin_ctx = tc.tile_pool(name="tin", bufs=1)
            tin = tin_ctx.__enter__()
            qT3 = tin.tile([128, ND, SQ], BF16, tag="qT3", name="qT3")
            nc.sync.dma_start_transpose(qT3, q_d[:, :])
            kT3 = tin.tile([128, ND, SKV], BF16, tag="kT3", name="kT3")
            nc.sync.dma_start_transpose(kT3, k_d[:, :])
            vT3 = tin.tile([128, ND, SKV], BF16, tag="vT3", name="vT3")
            nc.sync.dma_start_transpose(vT3, v_d[:, :])
            wqt_sb = [tin.tile([128, D], BF16, tag=f"wqt{i}", name=f"wqt{i}") for i in range(ND)]
            wkt_sb = [tin.tile([128, D], BF16, tag=f"wkt{i}", name=f"wkt{i}") for i in range(ND)]
            wvt_sb = [tin.tile([128, D], BF16, tag=f"wvt{i}", name=f"wvt{i}") for i in range(ND)]
            for i in range(ND):
                nc.sync.dma_start(out=wqt_sb[i], in_=wqt_d[i * 128:(i + 1) * 128, :])
                nc.sync.dma_start(out=wkt_sb[i], in_=wkt_d[i * 128:(i + 1) * 128, :])
                nc.sync.dma_start(out=wvt_sb[i], in_=wvt_d[i * 128:(i + 1) * 128, :])

            mask_done = [False] * NQ

            def build_mask(qt):
                if mask_done[qt]:
                    return
                mask_done[qt] = True
                aff_t = mld.tile([128, SKV], BF16, tag="aff", name="aff")
                nc.gpsimd.dma_start(out=aff_t, in_=aff_d[qt * 128:(qt + 1) * 128, :])
                qbz_t = mld.tile([128, SKV], BF16, tag="qbz", name="qbz")
                nc.gpsimd.dma_start(out=qbz_t, in_=qbz_d[qt * 128:(qt + 1) * 128, :])
                eng = nc.vector if qt % 2 == 0 else nc.gpsimd
                eng.tensor_tensor(out=zb[qt], in0=qbz_t, in1=kvzb, op=Alu.mult)
                eng2 = nc.gpsimd if qt % 2 == 0 else nc.vector
                eng2.tensor_tensor(out=mm[qt], in0=aff_t, in1=zb[qt], op=Alu.mult)
                zcnt = mtmp.tile([128, 1], F32, tag="zcnt", name="zcnt")
                nc.vector.tensor_reduce(
                    out=zcnt, in_=zb[qt], axis=mybir.AxisListType.X, op=Alu.add
                )
                nc.vector.tensor_scalar(
                    out=negm[qt], in0=zcnt, scalar1=float(-SKV), scalar2=None,
                    op0=Alu.add,
                )

            # ---- phase B: projections ---------------------------------------
            with tc.tile_pool(name="pps", bufs=2, space="PSUM") as pps:
                def evict(dst, ps, bias_col):
                    if zero_bias:
                        nc.scalar.copy(out=dst, in_=ps)
                    else:
                        nc.vector.tensor_scalar(
                            out=dst, in0=ps, scalar1=bias_col, scalar2=None,
                            op0=Alu.add,
                        )

                for ot in range(ND):
                    ps = pps.tile([128, SQ], F32, tag="pq", name="pq")
                    for it in range(ND):
                        nc.tensor.matmul(
                            ps,
                            wqt_sb[it][:, ot * 128:(ot + 1) * 128],
                            qT3[:, it, :],
                            start=(it == 0),
                            stop=(it == ND - 1),
                        )
                    evict(qpt[ot], ps, None if zero_bias else bq_sb[:, ot:ot + 1])

                for ot in range(ND):
                    for kc in range(NKC):
                        ps = pps.tile([128, 512], F32, tag="pk", name="pk")
                        for it in range(ND):
                            nc.tensor.matmul(
                                ps,
                                wkt_sb[it][:, ot * 128:(ot + 1) * 128],
                                kT3[:, it, kc * 512:(kc + 1) * 512],
                                start=(it == 0),
                                stop=(it == ND - 1),
                            )
                        evict(kpt[ot][:, kc * 512:(kc + 1) * 512], ps,
                              None if zero_bias else bk_sb[:, ot:ot + 1])

                for kt in range(NK):
                    ps = pps.tile([128, D], F32, tag="pv", name="pv")
                    for it in range(ND):
                        nc.tensor.matmul(
                            ps,
                            vT3[:, it, kt * 128:(kt + 1) * 128],
                            wvt_sb[it],
                            start=(it == 0),
                            stop=(it == ND - 1),
                        )
                    if zero_bias:
                        nc.scalar.copy(out=vpa[kt][:, :, :].rearrange("p h d -> p (h d)"), in_=ps)
                    else:
                        nc.vector.tensor_tensor(
                            out=vpa[kt][:, :, :],
                            in0=ps.rearrange("p (h d) -> p h d", h=H),
                            in1=bv_sb.rearrange("p (h d) -> p h d", h=H),
                            op=Alu.add,
                        )

            tin_ctx.__exit__(None, None, None)

            # ---- phase 3: attention main loop -------------------------------
            with tc.tile_pool(name="sps", bufs=3, space="PSUM") as sps, \
                 tc.tile_pool(name="ops", bufs=1, space="PSUM") as ops, \
                 tc.tile_pool(name="e1p", bufs=3) as e1p, \
                 tc.tile_pool(name="ebp", bufs=3) as ebp, \
                 tc.tile_pool(name="etp", bufs=3) as etp, \
                 tc.tile_pool(name="wp", bufs=2) as wp, \
                 tc.tile_pool(name="srp", bufs=4) as srp:
                for hp in range(H // 2):
                    h0, h1 = 2 * hp, 2 * hp + 1
                    eTs = {
                        h0: etp.tile([128, NK, NQ, 128], BF16, tag="eT", name="eT0"),
                        h1: etp.tile([128, NK, NQ, 128], BF16, tag="eT", name="eT1"),
                    }
                    for qt in range(NQ):
                        build_mask(qt)
                        e1s = {}
                        for h in (h0, h1):
                            e1s[h] = e1p.tile([128, SKV], F32, tag="e1",
                                              name="e1")
                        # interleave the two heads' matmuls so they occupy
                        # different PE row groups concurrently
                        for kc in range(NKC):
                            for h in (h0, h1):
                                row0 = 64 * (h % 2)
                                ps = sps.tile([128, 512], F32, tag=f"s{h % 2}",
                                              name="ps")
                                nc.tensor.matmul(
                                    ps,
                                    qpt[hp][row0:row0 + 64,
                                            qt * 128:(qt + 1) * 128],
                                    kpt[hp][row0:row0 + 64,
                                            kc * 512:(kc + 1) * 512],
                                    start=True,
                                    stop=True,
                                )
                                nc.vector.tensor_tensor(
                                    out=e1s[h][:, kc * 512:(kc + 1) * 512],
                                    in0=ps,
                                    in1=mm[qt][:, kc * 512:(kc + 1) * 512],
                                    op=Alu.mult,
                                )
                        for h in (h0, h1):
                            e1 = e1s[h]
                            row0 = 64 * (h % 2)
                            e_bf = ebp.tile([128, SKV], BF16, tag="eb",
                                            name="e_bf")
                            rparts = []
                            for ec in range(2):
                                rp = srp.tile([128, 1], F32, tag="rp", name="rp")
                                nc.scalar.activation(
                                    out=e_bf[:, ec * 1024:(ec + 1) * 1024],
                                    in_=e1[:, ec * 1024:(ec + 1) * 1024],
                                    func=Act.Exp, accum_out=rp,
                                )
                                rparts.append(rp)
                            r01 = srp.tile([128, 1], F32, tag="r01", name="r01")
                            nc.vector.tensor_tensor(
                                out=r01, in0=rparts[0], in1=rparts[1], op=Alu.add
                            )
                            rt = srp.tile([128, 1], F32, tag="rt", name="rt")
                            nc.vector.tensor_tensor(
                                out=rt, in0=r01, in1=negm[qt], op=Alu.add
                            )
                            rc = srp.tile([128, 1], F32, tag="rc", name="rc")
                            nc.vector.reciprocal(out=rc, in_=rt)
                            # mask zeros back in (exp(0)=1 at masked spots)
                            e_z = ebp.tile([128, SKV], BF16, tag="ez",
                                           name="e_z")
                            if h % 2 == 0:
                                nc.vector.tensor_tensor(
                                    out=e_z, in0=e_bf, in1=zb[qt], op=Alu.mult
                                )
                            else:
                                nc.gpsimd.tensor_tensor(
                                    out=e_z, in0=e_bf, in1=zb[qt], op=Alu.mult
                                )
                            wt = wp.tile([128, SKV], BF16, tag="w",
                                         name="wt")
                            nc.vector.tensor_scalar(
                                out=wt, in0=e_z, scalar1=rc, scalar2=None,
                                op0=Alu.mult,
                            )
                            nc.gpsimd.dma_start(
                                out=w_out[h, qt * 128:(qt + 1) * 128, :], in_=wt
                            )
                            nc.sync.dma_start_transpose(eTs[h][:, :, qt, :], wt)

                    for h in (h0, h1):
                        row0 = 64 * (h % 2)
                        pso = ops.tile([DH, SQ], F32, tag=f"pv{h % 2}", name="pso")
                        for kt in range(NK):
                            nc.tensor.matmul(
                                pso,
                                vpa[kt][:, h, :],
                                eTs[h][:, kt, :, :],
                                start=(kt == 0),
                                stop=(kt == NK - 1),
                            )
                        nc.scalar.copy(out=att[hp][row0:row0 + 64, :], in_=pso)

            mtmp_ctx.__exit__(None, None, None)
            mld_ctx.__exit__(None, None, None)

            # ---- output projection ------------------------------------------
            with tc.tile_pool(name="oop", bufs=2, space="PSUM") as oop, \
                 tc.tile_pool(name="wp2", bufs=2) as wp2:
                for qt in range(NQ):
                    ps = oop.tile([128, DO], F32, tag="oo")
                    for ct in range(ND):
                        nc.tensor.matmul(
                            ps,
                            att[ct][:, qt * 128:(qt + 1) * 128],
                            wot_sb[ct],
                            start=(ct == 0),
                            stop=(ct == ND - 1),
                        )
                    osb = wp2.tile([128, DO], F32, tag="osb")
                    if zero_bias:
                        nc.scalar.copy(out=osb, in_=ps)
                    else:
                        nc.vector.tensor_tensor(out=osb, in0=ps, in1=bo_sb, op=Alu.add)
                    nc.sync.dma_start(
                        out=o_out[qt * 128:(qt + 1) * 128, :], in_=osb
                    )

    _fix_excess_waits(nc)
    return nc


_NC_CACHE = {}


def _get_program(zero_bias=True):
    key = ("nc", zero_bias)
    if key not in _NC_CACHE:
        _NC_CACHE[key] = build_program(zero_bias)
    return _NC_CACHE[key]


def _prep_in_maps(Q, K, V, q_aff_mask, q_binary_mask, kv_mask,
                  Wq, bq, Wk, bk, Wv, bv, Wo, bo):
    bf = ml_dtypes.bfloat16
    wqt = np.ascontiguousarray((Wq.astype(np.float32) * SCALE).T).astype(bf)
    wkt = np.ascontiguousarray(Wk.astype(np.float32).T).astype(bf)
    wvt = np.ascontiguousarray(Wv.astype(np.float32).T).astype(bf)
    wot = np.ascontiguousarray(Wo.astype(np.float32).T).astype(bf)
    bqs = np.ascontiguousarray((bq.astype(np.float32) * SCALE).reshape(4, 128).T)
    bkf = np.ascontiguousarray(bk.astype(np.float32).reshape(4, 128).T)
    bvf = bv.astype(np.float32).reshape(1, D)
    bof = bo.astype(np.float32).reshape(1, DO)

    in_maps = []
    for c in range(N_CORES):
        b, qh = c // 2, c % 2
        qs = slice(qh * SQ, (qh + 1) * SQ)
        in_maps.append({
            "q": Q[b, qs, :].astype(bf),
            "k": K[b].astype(bf),
            "v": V[b].astype(bf),
            "aff": q_aff_mask[b, qs, :].astype(bf),
            "qbz": (1 - q_binary_mask[b, qs, :]).astype(bf),
            "kvz": (1 - kv_mask[b, :, 0]).astype(bf).reshape(1, SKV),
            "wqt": wqt, "wkt": wkt, "wvt": wvt, "wot": wot,
            "bq": bqs, "bk": bkf, "bv": bvf, "bo": bof,
        })
    return in_maps


def run(in_maps, trace=False, zero_bias=True, **kw):
    nc = _get_program(zero_bias)
    return run_bass_kernel_spmd(nc, in_maps, list(range(N_CORES)), trace=trace, **kw)


def _all_zero_bias(inputs):
    return all(
        not np.any(np.asarray(inputs[k])) for k in ("bq", "bk", "bv", "bo")
    )


def kernel(**inputs):
    in_maps = _prep_in_maps(**inputs)
    zb_flag = _all_zero_bias(inputs)
    res = run(in_maps, zero_bias=zb_flag)
    att_output = np.empty((B, SQ_FULL, DO), np.float32)
    att_weights = np.empty((B, H, SQ_FULL, SKV), np.float32)
    for c in range(N_CORES):
        b, qh = c // 2, c % 2
        qs = slice(qh * SQ, (qh + 1) * SQ)
        att_output[b, qs, :] = res.results[c]["o_out"]
        att_weights[b, :, qs, :] = res.results[c]["w_out"].astype(np.float32)
    return att_output, att_weights


if __name__ == "__main__":
    t0 = time.time()
    _get_program()
    print("build s:", time.time() - t0)
